# revision 18
# baseline (speedup 1.0000x reference)
"""MoE (MiMo-V2) kernel for 8x Trainium2 NeuronCores.

Strategy (expert-parallel with expert splitting):
  - Host: grouped-topk routing (exact replica of the reference gate, fp32 on
    jax-cpu). Experts are split into pieces (a piece = contiguous run of one
    expert's tokens); pieces are rank-grouped 8-at-a-time into SPMD slots with
    shared caps. Piece sizes are optimized (structured init + hill climb) to
    minimize PE cycles: f = 96*sum(caps) + 6144*sum(ceil(cap/128)), i.e.
    gate/up columns plus fixed-cost 128-token down tiles. Splitting lets caps
    hug the counts (padding ~1%o vs ~2.7% for whole-expert assignment).
  - Device (Bass/Tile, one SPMD program): per slot, stream token blocks of
    <=1024 through gate/up matmuls (bf16, fp32 PSUM, explicit LDWEIGHTS with
    weight-elided matmuls), silu*mul on ACT/DVE into an [I, tokens] act tile;
    the previous block's down matmuls are interleaved between gate/up chunk
    pairs; down rows are scaled by combine weights and written out in bf16.
    Weights are stored phase-major ([12, 128, 1024] per expert) so the first
    matmul needs only one 256KB transfer; each x block is a single 3D-AP DMA
    (the DMA-trigger issue rate on the sync sequencer, ~0.6us each, is a
    warmup bottleneck). Slots are ordered big/small interleaved so small
    slots' weights prefetch under big slots' compute.
  - Host: scatter-add the per-piece rows into the [T, H] output.
"""

import numpy as np
import ml_dtypes

T, H, E, I, K, G, KG = 16384, 1024, 64, 768, 8, 8, 4
P = 128
NCORES = 8
HC = H // P  # 8 contraction chunks for gate/up
IC = I // P  # 6 contraction chunks for down (also the jj phase count)
NPH = 2 * IC  # 12 gate/up phases (gate jj, up jj)
BLK = 1024  # token block (<=2 PSUM sub-blocks of 512 per phase)
MAXTILES = 8  # down tiles per slot (cap <= 1024)

BF16 = ml_dtypes.bfloat16

_program_cache = {}
_weights_cache = {}
last_results = None  # BassKernelResults of the most recent launch (for test.py)


def _routing_np(hidden, gate_w, bias):
    """Numpy fallback for the grouped-topk gate (same ops/tie rules)."""
    logits = hidden.astype(np.float32) @ gate_w.T.astype(np.float32)
    scores = 1.0 / (1.0 + np.exp(-logits))
    s_choice = scores + bias[None, :].astype(np.float32)
    t, e = scores.shape
    grouped = s_choice.reshape(t, G, e // G)
    top2 = np.sort(grouped, axis=-1)[..., -2:]
    group_scores = top2.sum(-1)
    gidx = np.argsort(-group_scores, axis=1, kind="stable")[:, :KG]
    gmask = np.zeros((t, G), np.float32)
    gmask[np.arange(t)[:, None], gidx] = 1.0
    emask = np.repeat(gmask, e // G, axis=1)
    masked = np.where(emask > 0, s_choice, -np.inf)
    topk_idx = np.argsort(-masked, axis=1, kind="stable")[:, :K].astype(np.int32)
    topk_w = np.take_along_axis(scores, topk_idx, axis=1)
    topk_w = topk_w / (topk_w.sum(-1, keepdims=True) + 1e-20)
    return topk_idx, topk_w.astype(np.float32)


def _routing(hidden, gate_w, bias):
    """Exact replica of reference._grouped_topk on jax-cpu (fp32)."""
    try:
        import jax
        import jax.numpy as jnp

        cpu = jax.devices("cpu")[0]
    except Exception:
        return _routing_np(np.asarray(hidden), np.asarray(gate_w), np.asarray(bias))
    with jax.default_device(cpu):
        hidden = jnp.asarray(np.asarray(hidden), jnp.float32)
        gate_w = jnp.asarray(np.asarray(gate_w), jnp.float32)
        bias = jnp.asarray(np.asarray(bias), jnp.float32)
        logits = hidden @ gate_w.T
        scores = jax.nn.sigmoid(logits)
        s_choice = scores + bias[None, :]
        t, e = scores.shape
        grouped = s_choice.reshape(t, G, e // G)
        top2, _ = jax.lax.top_k(grouped, 2)
        group_scores = top2.sum(-1)
        _, gidx = jax.lax.top_k(group_scores, KG)
        gmask = jnp.zeros((t, G), jnp.float32).at[jnp.arange(t)[:, None], gidx].set(1.0)
        emask = jnp.repeat(gmask, e // G, axis=1)
        masked = jnp.where(emask > 0, s_choice, -jnp.inf)
        _, topk_idx = jax.lax.top_k(masked, K)
        topk_w = jnp.take_along_axis(scores, topk_idx, axis=1)
        topk_w = topk_w / (topk_w.sum(-1, keepdims=True) + 1e-20)
        return np.asarray(topk_idx), np.asarray(topk_w, np.float32)


def _pack_pieces(counts, iters=80000, seed=1):
    """Split expert counts into pieces so that the sorted piece multiset
    rank-groups (8 at a time) into slots with minimal PE cost.

    Cost per core: f = 96*sum(caps) + 6144*sum(ceil(caps/128)) cycles
    (gate/up columns + fixed-size 128-token down tiles). Structured init:
    one ragged piece of 384+(c%128) per expert (ragged caps cluster by
    c%128), remainder in 128-multiple pieces of <=1024. A hill climb with
    128-granular and fine moves between same-expert pieces cleans up.
    Returns eps: list (per expert) of piece sizes.
    """
    import random

    rng = random.Random(seed)
    counts = [int(c) for c in counts]
    eps = []
    for c in counts:
        if c <= 0:
            eps.append([])
            continue
        if c < 896:
            eps.append([c])
            continue
        rag = 384 + (c % 128)
        n = (c - rag) // 128
        ps = [rag]
        while n > 8:
            take = min(8, n - 4)
            ps.append(128 * take)
            n -= take
        if n:
            ps.append(128 * n)
        eps.append(ps)

    def nflat():
        return sum(len(ps) for ps in eps)

    while nflat() % 8:
        bi, bj = max(
            ((i, j) for i, ps in enumerate(eps) for j in range(len(ps))),
            key=lambda t: eps[t[0]][t[1]],
        )
        p = eps[bi].pop(bj)
        h = max(128, (p // 2) // 128 * 128)
        eps[bi] += [p - h, h]

    def obj():
        flat = sorted((p for ps in eps for p in ps), reverse=True)
        caps = flat[0::8]
        fv = 96 * sum(caps) + 6144 * sum(-(-c // 128) for c in caps)
        if flat[-1] < 384:
            fv += 1_000_000
        return fv

    cur = obj()
    movers = [i for i, ps in enumerate(eps) if len(ps) >= 2]
    if movers:
        for _ in range(iters):
            i = movers[rng.randrange(len(movers))]
            ps = eps[i]
            a = rng.randrange(len(ps))
            b = rng.randrange(len(ps))
            if a == b:
                continue
            d = rng.choice((1, 2, 4, 8, 16, 32, 64, 128, 256))
            if ps[a] - d < 128 or ps[b] + d > BLK:
                continue
            ps[a] -= d
            ps[b] += d
            f2 = obj()
            if f2 <= cur:
                cur = f2
            else:
                ps[a] += d
                ps[b] -= d
    return eps


def _blocks_of(cap, warm=False):
    """Decompose a slot capacity into token blocks of <=BLK.

    All blocks except the last are multiples of 128 (keeps down tiles
    128-aligned within the slot). warm=True (first slot) starts with a
    512-token block for a short first-weights transfer and early PE ramp.
    """
    bl = []
    work = cap
    if warm and cap >= 896:
        bl.append(256)
        work -= 256
        if work > BLK:
            bl.append(768)
            work -= 768
    nb = -(-work // BLK)
    base = int(round(work / nb / P)) * P
    while work - base * (nb - 1) > BLK:
        base += P
    while nb > 1 and work - base * (nb - 1) <= 0:
        base -= P
    bl += [base] * (nb - 1) + [work - base * (nb - 1)]
    assert all(0 < b <= BLK for b in bl) and sum(bl) == cap, (cap, bl)
    return bl


def _build_program(slot_blocks):
    """One SPMD Bass program. slot_blocks[i] is the token-block decomposition
    of slot i (fixed caps shared by all cores)."""
    import concourse.mybir as mybir
    from concourse import bacc
    from concourse.tile import TileContext

    caps = [sum(b) for b in slot_blocks]
    m = len(caps)
    seg_off = np.zeros(m + 1, np.int64)
    np.cumsum(caps, out=seg_off[1:])
    NC = int(seg_off[-1])
    bf = mybir.dt.bfloat16
    f32 = mybir.dt.float32
    Silu = mybir.ActivationFunctionType.Silu
    mult = mybir.AluOpType.mult

    # All bulk inputs are host-packed partition-major so every DMA is 128
    # contiguous runs (descriptor issue on the sequencer costs ~4.7ns/run;
    # multi-KB-per-partition transfers keep issue at ~0.6us each).
    blk_off = []  # [slot][block] -> column offset into xq
    xcols = 0
    for bl in slot_blocks:
        offs = []
        for bn in bl:
            offs.append(xcols)
            xcols += HC * bn
        blk_off.append(offs)

    nc = bacc.Bacc("TRN2", target_bir_lowering=False, debug=False, num_devices=NCORES)
    xq = nc.dram_tensor("xq", [P, xcols], bf, kind="ExternalInput").ap()
    wgu = nc.dram_tensor("wgu", [m, P, NPH * H], bf, kind="ExternalInput").ap()
    wd = nc.dram_tensor("wd", [m, P, IC * H], bf, kind="ExternalInput").ap()
    cv = nc.dram_tensor("cv", [m, P, MAXTILES], f32, kind="ExternalInput").ap()
    g = nc.dram_tensor("g", [NC, H], bf, kind="ExternalOutput").ap()

    def mm_group(tensor_eng, w_ap, mms):
        """Explicit LDWEIGHTS + weight-elided matmuls sharing it."""
        tensor_eng.ldweights(w_ap)
        for out_ap, rhs_ap, start, stop in mms:
            mm = tensor_eng.matmul(
                out=out_ap, lhsT=w_ap, rhs=rhs_ap, start=start, stop=stop
            )
            mm.ins.ldweights = False

    with TileContext(nc) as tc:
        with (
            tc.tile_pool(name="wpool", bufs=2) as wpool,
            tc.tile_pool(name="xpool", bufs=3) as xpool,
            tc.tile_pool(name="apool", bufs=2) as apool,
            tc.tile_pool(name="spool", bufs=2) as spool,
            tc.tile_pool(name="opool", bufs=6) as opool,
            tc.tile_pool(name="cpool", bufs=2) as cpool,
            tc.tile_pool(name="psgu", bufs=1, space="PSUM") as psgu,
            tc.tile_pool(name="pso", bufs=2, space="PSUM") as pso,
        ):
            wgu_r = wgu.rearrange("m p (f h) -> m p f h", f=NPH)  # [m, 128, NPH, H]
            wd_r = wd.rearrange("m p (c h) -> m p c h", c=IC)  # [m, 128, IC, H]
            pending_down = []

            def _make_down_tile(gq, q, t0, tn, act_tile, wd_tile, ct_tile):
                # gq: global output row base; q: slot-local tile index (cv col);
                # t0: token offset within the act tile's block
                def emit():
                    po = pso.tile([P, H], f32, tag="po", name="po")
                    for i in range(IC):
                        mm_group(
                            nc.tensor,
                            act_tile[:, i, t0 : t0 + tn],
                            [
                                (
                                    po[:tn, nh * 512 : (nh + 1) * 512],
                                    wd_tile[:, i, nh * 512 : (nh + 1) * 512],
                                    i == 0,
                                    i == IC - 1,
                                )
                                for nh in range(2)
                            ],
                        )
                    ob = opool.tile([P, H], bf, tag="ob", name="ob")
                    nc.vector.tensor_tensor(
                        out=ob[:tn, :],
                        in0=po[:tn, :],
                        in1=ct_tile[:tn, q : q + 1].to_broadcast([tn, H]),
                        op=mult,
                    )
                    # NOTE: must issue on sync, not scalar — a dependent DMA
                    # on the scalar queue blocks later silu instructions
                    # behind its semaphore wait and stalls the PE
                    nc.sync.dma_start(out=g[gq : gq + tn, :], in_=ob[:tn, :])

                return emit

            for ei in range(m):
                blocks = slot_blocks[ei]
                wgu_t = wpool.tile([P, NPH, H], bf, tag="wgu")
                if ei == 0:
                    # head latency: per-queue DMA bw is ~22GB/s, so the first
                    # x block (256 tokens, 512KB) and phase-0 weights (256KB)
                    # arrive ~2.5us after issue; later phases stream in chunks
                    # sized to land just before their compute
                    nc.sync.dma_start(out=wgu_t[:, 0, :], in_=wgu_r[0][:, 0, :])
                    bn0 = blocks[0]
                    xg_t = xpool.tile([P, HC * BLK], bf, tag="xg")
                    hx = HC * bn0 // 2
                    nc.sync.dma_start(out=xg_t[:, :hx], in_=xq[:, 0:hx])
                    nc.sync.dma_start(
                        out=xg_t[:, hx : HC * bn0], in_=xq[:, hx : HC * bn0]
                    )
                    nc.sync.dma_start(out=wgu_t[:, 1:3, :], in_=wgu_r[0][:, 1:3, :])
                    nc.sync.dma_start(out=wgu_t[:, 3:6, :], in_=wgu_r[0][:, 3:6, :])
                    nc.sync.dma_start(out=wgu_t[:, 6:, :], in_=wgu_r[0][:, 6:, :])
                    # prewarm: the PE clock boost needs ~3us of continuous
                    # execution (and resets on idle); run dummy matmuls on the
                    # just-arrived phase-0 weights while block-0 x is still in
                    # flight so real matmuls start at full clock
                    pwarm = pso.tile([P, H], f32, tag="po", name="po")
                    for _ in range(12):
                        mm_group(
                            nc.tensor,
                            wgu_t[:, 0, 0:P],
                            [(pwarm[:, :512], wgu_t[:, 0, 0:512], True, True)],
                        )
                else:
                    nc.sync.dma_start(out=wgu_t[:], in_=wgu_r[ei])
                wd_t = wpool.tile([P, IC, H], bf, tag="wd")
                nc.sync.dma_start(out=wd_t[:], in_=wd_r[ei])
                ct = cpool.tile([P, MAXTILES], f32, tag="ct")
                nc.sync.dma_start(out=ct[:], in_=cv[ei])

                off = 0
                for bi, bn in enumerate(blocks):
                    s = int(seg_off[ei]) + off
                    if not (ei == 0 and bi == 0):
                        xg_t = xpool.tile([P, HC * BLK], bf, tag="xg")
                        bo = blk_off[ei][bi]
                        nc.sync.dma_start(
                            out=xg_t[:, : HC * bn], in_=xq[:, bo : bo + HC * bn]
                        )
                    # sub-blocks of <=512, smallest first: the LAST matmul of
                    # each weight group must be wide enough to hide the next
                    # group's LDWEIGHTS behind its streaming
                    sbs = sorted(
                        (
                            (qq * 512, min(512, bn - qq * 512))
                            for qq in range((bn + 511) // 512)
                        ),
                        key=lambda t: t[1],
                    )
                    act_sb = apool.tile([P, IC, BLK], bf, tag="act")
                    # gate/up phase pairs with the previous block's down tiles
                    # interleaved between pairs (stretches every PSUM-reuse
                    # distance past the ACT/DVE consumer chain)
                    ndp = len(pending_down)
                    emitted = 0
                    for jj in range(IC):
                        pg = [
                            psgu.tile([P, 512], f32, tag=f"pg{si}", name=f"pg{si}")
                            for si in range(len(sbs))
                        ]
                        pu = [
                            psgu.tile([P, 512], f32, tag=f"pu{si}", name=f"pu{si}")
                            for si in range(len(sbs))
                        ]
                        for gi, ps_tiles in ((0, pg), (1, pu)):
                            ph = 2 * jj + gi
                            for hc in range(HC):
                                mm_group(
                                    nc.tensor,
                                    wgu_t[:, ph, hc * P : (hc + 1) * P],
                                    [
                                        (
                                            ps_tiles[si][:, :qn],
                                            xg_t[:, hc * bn + q0 : hc * bn + q0 + qn],
                                            hc == 0,
                                            hc == HC - 1,
                                        )
                                        for si, (q0, qn) in enumerate(sbs)
                                    ],
                                )
                        for si, (q0, qn) in enumerate(sbs):
                            sg = spool.tile([P, 512], f32, tag=f"sg{si}", name=f"sg{si}")
                            nc.scalar.activation(
                                out=sg[:, :qn], in_=pg[si][:, :qn], func=Silu
                            )
                            nc.vector.tensor_tensor(
                                out=act_sb[:, jj, q0 : q0 + qn],
                                in0=sg[:, :qn],
                                in1=pu[si][:, :qn],
                                op=mult,
                            )
                        target = (jj + 1) * ndp // IC
                        while emitted < target:
                            pending_down.pop(0)()
                            emitted += 1
                    nt = (bn + P - 1) // P
                    for ts in range(nt):
                        pending_down.append(
                            _make_down_tile(
                                s + ts * P,
                                (off + ts * P) // P,
                                ts * P,
                                min(P, bn - ts * P),
                                act_sb,
                                wd_t,
                                ct,
                            )
                        )
                    off += bn
            while pending_down:
                pending_down.pop(0)()
    nc.compile()
    return nc


def kernel(hidden_states, gate_weight, correction_bias, w_gate, w_up, w_down):
    global last_results
    from concourse.bass_utils import run_bass_kernel_spmd

    hidden = np.ascontiguousarray(np.asarray(hidden_states, np.float32))
    w_gate = np.asarray(w_gate, np.float32)
    w_up = np.asarray(w_up, np.float32)
    w_down = np.asarray(w_down, np.float32)

    topk_idx, topk_w = _routing(hidden, gate_weight, correction_bias)

    # Per-expert token lists (ascending), via stable sort of the (token, k) pairs.
    flat_e = topk_idx.ravel()
    order = np.argsort(flat_e, kind="stable")
    tokens_sorted = (order // K).astype(np.int64)
    weights_sorted = topk_w.ravel()[order]
    counts = np.bincount(flat_e, minlength=E)
    starts = np.zeros(E + 1, np.int64)
    np.cumsum(counts, out=starts[1:])

    # Pieces -> rank groups of 8 -> slots; big/small interleaved slot order.
    eps = _pack_pieces(counts)
    pieces = []  # (size, expert, offset within expert token list)
    for e, ps in enumerate(eps):
        off = 0
        for p in sorted(ps, reverse=True):
            pieces.append((int(p), e, off))
            off += p
    pieces.sort(key=lambda t: (-t[0], t[1], t[2]))
    assert len(pieces) % 8 == 0
    nrank = len(pieces) // 8
    half = (nrank + 1) // 2
    rank_order = []
    for i in range(half):
        rank_order.append(i)
        if i + half < nrank:
            rank_order.append(i + half)
    # slot i <- rank rank_order[i]
    slot_pieces = [pieces[8 * r : 8 * r + 8] for r in rank_order]
    caps = [grp[0][0] for grp in slot_pieces]
    m = len(caps)
    slot_blocks = tuple(
        tuple(_blocks_of(int(caps[i]), warm=(i == 0))) for i in range(m)
    )
    seg_off = np.zeros(m + 1, np.int64)
    np.cumsum(caps, out=seg_off[1:])
    NC = int(seg_off[-1])

    print(
        f"[kernel] counts min/mean/max: {counts.min()}/{counts.mean():.0f}/{counts.max()}; "
        f"m={m} sumcaps {NC} pad {8 * NC - int(counts.sum())}"
    )
    if slot_blocks not in _program_cache:
        _program_cache[slot_blocks] = _build_program([list(b) for b in slot_blocks])
    nc = _program_cache[slot_blocks]

    # per-expert phase-major weight arrays (cached across calls by id fingerprint)
    wkey = (
        float(w_gate[0, 0, 0]),
        float(w_up[0, 0, 0]),
        float(w_down[-1, -1, -1]),
        w_gate.shape,
    )
    cached = _weights_cache.get(wkey)
    if cached is None:
        # partition-major: wgu_e[e, p, ph*H + hc*128 + c], ph = 2*jj + (0 g/1 u)
        wgu_e = np.empty((E, P, NPH, H), BF16)
        wd_e = np.empty((E, P, IC, H), BF16)
        for e in range(E):
            gp = (
                w_gate[e].reshape(IC, P, HC, P).transpose(3, 0, 2, 1).reshape(P, IC, H)
            )  # [p, jj, hc*128+c] = wg[jj*128+c, hc*128+p]
            up = w_up[e].reshape(IC, P, HC, P).transpose(3, 0, 2, 1).reshape(P, IC, H)
            wgu_e[e, :, 0::2] = gp.astype(BF16)
            wgu_e[e, :, 1::2] = up.astype(BF16)
            # wd_e[e, p, ic*H + h] = w_down[e][h, ic*128+p]
            wd_e[e] = (
                w_down[e].T.reshape(IC, P, H).transpose(1, 0, 2).astype(BF16)
            )
        wgu_e = wgu_e.reshape(E, P, NPH * H)
        wd_e = wd_e.reshape(E, P, IC * H)
        _weights_cache.clear()
        cached = (wgu_e, wd_e)
        _weights_cache[wkey] = cached
    wgu_e, wd_e = cached

    # x block layout: per (slot, block), partition-major [128, HC*bn] columns
    blk_off = []
    xcols = 0
    for bl in slot_blocks:
        offs = []
        for bn in bl:
            offs.append(xcols)
            xcols += HC * bn
        blk_off.append(offs)

    hidden_bf_t = np.ascontiguousarray(hidden.T).astype(BF16)  # [H, T]
    hbt3 = hidden_bf_t.reshape(HC, P, T)
    in_maps = []
    core_slot_info = []  # [core][slot] = (expert, n, token array)
    for c in range(NCORES):
        slot_exp = np.empty(m, np.int64)
        perm = np.zeros(NC, np.int64)
        cvh = np.zeros((m, P, MAXTILES), np.float32)
        info = []
        for i, grp in enumerate(slot_pieces):
            pn, e, poff = grp[c]
            slot_exp[i] = e
            s = int(seg_off[i])
            te = tokens_sorted[starts[e] + poff : starts[e] + poff + pn]
            perm[s : s + pn] = te
            wv = weights_sorted[starts[e] + poff : starts[e] + poff + pn]
            wpad = np.zeros(P * MAXTILES, np.float32)
            wpad[:pn] = wv
            cvh[i] = wpad.reshape(MAXTILES, P).T
            info.append((e, pn, te))
        core_slot_info.append(info)
        xqc = np.empty((P, xcols), BF16)
        for i in range(m):
            s = int(seg_off[i])
            off = 0
            for bi, bn in enumerate(slot_blocks[i]):
                bo = blk_off[i][bi]
                blkx = hbt3[:, :, perm[s + off : s + off + bn]]  # [HC, P, bn]
                xqc[:, bo : bo + HC * bn] = blkx.transpose(1, 0, 2).reshape(
                    P, HC * bn
                )
                off += bn
        in_maps.append(
            {
                "xq": xqc,
                "wgu": wgu_e[slot_exp],
                "wd": wd_e[slot_exp],
                "cv": cvh,
            }
        )

    last_results = run_bass_kernel_spmd(nc, in_maps, list(range(NCORES)))

    out = np.zeros((T, H), np.float32)
    for c in range(NCORES):
        gc = last_results.results[c]["g"]
        for i in range(m):
            e, pn, te = core_slot_info[c][i]
            s = int(seg_off[i])
            out[te] += gc[s : s + pn].astype(np.float32)
    return out


# revision 19
# speedup vs baseline: 1.0045x; 1.0045x over previous
"""MoE (MiMo-V2) kernel for 8x Trainium2 NeuronCores.

Strategy (expert-parallel with expert splitting):
  - Host: grouped-topk routing (exact replica of the reference gate, fp32 on
    jax-cpu). Experts are split into pieces (a piece = contiguous run of one
    expert's tokens); pieces are rank-grouped 8-at-a-time into SPMD slots with
    shared caps. Piece sizes are optimized (structured init + hill climb) to
    minimize PE cycles: f = 96*sum(caps) + 6144*sum(ceil(cap/128)), i.e.
    gate/up columns plus fixed-cost 128-token down tiles. Splitting lets caps
    hug the counts (padding ~1%o vs ~2.7% for whole-expert assignment).
  - Device (Bass/Tile, one SPMD program): per slot, stream token blocks of
    <=1024 through gate/up matmuls (bf16, fp32 PSUM, explicit LDWEIGHTS with
    weight-elided matmuls), silu*mul on ACT/DVE into an [I, tokens] act tile;
    the previous block's down matmuls are interleaved between gate/up chunk
    pairs; down rows are scaled by combine weights and written out in bf16.
    Weights are stored phase-major ([12, 128, 1024] per expert) so the first
    matmul needs only one 256KB transfer; each x block is a single 3D-AP DMA
    (the DMA-trigger issue rate on the sync sequencer, ~0.6us each, is a
    warmup bottleneck). Slots are ordered big/small interleaved so small
    slots' weights prefetch under big slots' compute.
  - Host: scatter-add the per-piece rows into the [T, H] output.
"""

import numpy as np
import ml_dtypes

T, H, E, I, K, G, KG = 16384, 1024, 64, 768, 8, 8, 4
P = 128
NCORES = 8
HC = H // P  # 8 contraction chunks for gate/up
IC = I // P  # 6 contraction chunks for down (also the jj phase count)
NPH = 2 * IC  # 12 gate/up phases (gate jj, up jj)
BLK = 1024  # token block (<=2 PSUM sub-blocks of 512 per phase)
MAXTILES = 8  # down tiles per slot (cap <= 1024)

BF16 = ml_dtypes.bfloat16

_program_cache = {}
_weights_cache = {}
last_results = None  # BassKernelResults of the most recent launch (for test.py)


def _routing_np(hidden, gate_w, bias):
    """Numpy fallback for the grouped-topk gate (same ops/tie rules)."""
    logits = hidden.astype(np.float32) @ gate_w.T.astype(np.float32)
    scores = 1.0 / (1.0 + np.exp(-logits))
    s_choice = scores + bias[None, :].astype(np.float32)
    t, e = scores.shape
    grouped = s_choice.reshape(t, G, e // G)
    top2 = np.sort(grouped, axis=-1)[..., -2:]
    group_scores = top2.sum(-1)
    gidx = np.argsort(-group_scores, axis=1, kind="stable")[:, :KG]
    gmask = np.zeros((t, G), np.float32)
    gmask[np.arange(t)[:, None], gidx] = 1.0
    emask = np.repeat(gmask, e // G, axis=1)
    masked = np.where(emask > 0, s_choice, -np.inf)
    topk_idx = np.argsort(-masked, axis=1, kind="stable")[:, :K].astype(np.int32)
    topk_w = np.take_along_axis(scores, topk_idx, axis=1)
    topk_w = topk_w / (topk_w.sum(-1, keepdims=True) + 1e-20)
    return topk_idx, topk_w.astype(np.float32)


def _routing(hidden, gate_w, bias):
    """Exact replica of reference._grouped_topk on jax-cpu (fp32)."""
    try:
        import jax
        import jax.numpy as jnp

        cpu = jax.devices("cpu")[0]
    except Exception:
        return _routing_np(np.asarray(hidden), np.asarray(gate_w), np.asarray(bias))
    with jax.default_device(cpu):
        hidden = jnp.asarray(np.asarray(hidden), jnp.float32)
        gate_w = jnp.asarray(np.asarray(gate_w), jnp.float32)
        bias = jnp.asarray(np.asarray(bias), jnp.float32)
        logits = hidden @ gate_w.T
        scores = jax.nn.sigmoid(logits)
        s_choice = scores + bias[None, :]
        t, e = scores.shape
        grouped = s_choice.reshape(t, G, e // G)
        top2, _ = jax.lax.top_k(grouped, 2)
        group_scores = top2.sum(-1)
        _, gidx = jax.lax.top_k(group_scores, KG)
        gmask = jnp.zeros((t, G), jnp.float32).at[jnp.arange(t)[:, None], gidx].set(1.0)
        emask = jnp.repeat(gmask, e // G, axis=1)
        masked = jnp.where(emask > 0, s_choice, -jnp.inf)
        _, topk_idx = jax.lax.top_k(masked, K)
        topk_w = jnp.take_along_axis(scores, topk_idx, axis=1)
        topk_w = topk_w / (topk_w.sum(-1, keepdims=True) + 1e-20)
        return np.asarray(topk_idx), np.asarray(topk_w, np.float32)


def _pack_pieces(counts, iters=80000, seed=1):
    """Split expert counts into pieces so that the sorted piece multiset
    rank-groups (8 at a time) into slots with minimal PE cost.

    Cost per core: f = 96*sum(caps) + 6144*sum(ceil(caps/128)) cycles
    (gate/up columns + fixed-size 128-token down tiles). Structured init:
    one ragged piece of 384+(c%128) per expert (ragged caps cluster by
    c%128), remainder in 128-multiple pieces of <=1024. A hill climb with
    128-granular and fine moves between same-expert pieces cleans up.
    Returns eps: list (per expert) of piece sizes.
    """
    import random

    rng = random.Random(seed)
    counts = [int(c) for c in counts]
    eps = []
    for c in counts:
        if c <= 0:
            eps.append([])
            continue
        if c < 896:
            eps.append([c])
            continue
        rag = 384 + (c % 128)
        n = (c - rag) // 128
        ps = [rag]
        while n > 8:
            take = min(8, n - 4)
            ps.append(128 * take)
            n -= take
        if n:
            ps.append(128 * n)
        eps.append(ps)

    def nflat():
        return sum(len(ps) for ps in eps)

    while nflat() % 8:
        bi, bj = max(
            ((i, j) for i, ps in enumerate(eps) for j in range(len(ps))),
            key=lambda t: eps[t[0]][t[1]],
        )
        p = eps[bi].pop(bj)
        h = max(128, (p // 2) // 128 * 128)
        eps[bi] += [p - h, h]

    def obj():
        flat = sorted((p for ps in eps for p in ps), reverse=True)
        caps = flat[0::8]
        fv = 96 * sum(caps) + 6144 * sum(-(-c // 128) for c in caps)
        if flat[-1] < 384:
            fv += 1_000_000
        return fv

    cur = obj()
    movers = [i for i, ps in enumerate(eps) if len(ps) >= 2]
    if movers:
        for _ in range(iters):
            i = movers[rng.randrange(len(movers))]
            ps = eps[i]
            a = rng.randrange(len(ps))
            b = rng.randrange(len(ps))
            if a == b:
                continue
            d = rng.choice((1, 2, 4, 8, 16, 32, 64, 128, 256))
            if ps[a] - d < 128 or ps[b] + d > BLK:
                continue
            ps[a] -= d
            ps[b] += d
            f2 = obj()
            if f2 <= cur:
                cur = f2
            else:
                ps[a] += d
                ps[b] -= d
    return eps


def _blocks_of(cap, warm=False):
    """Decompose a slot capacity into token blocks of <=BLK.

    All blocks except the last are multiples of 128 (keeps down tiles
    128-aligned within the slot). warm=True (first slot) starts with a
    512-token block for a short first-weights transfer and early PE ramp.
    """
    bl = []
    work = cap
    if warm and cap >= 896:
        bl.append(256)
        work -= 256
        if work > BLK:
            bl.append(768)
            work -= 768
    nb = -(-work // BLK)
    base = int(round(work / nb / P)) * P
    while work - base * (nb - 1) > BLK:
        base += P
    while nb > 1 and work - base * (nb - 1) <= 0:
        base -= P
    bl += [base] * (nb - 1) + [work - base * (nb - 1)]
    assert all(0 < b <= BLK for b in bl) and sum(bl) == cap, (cap, bl)
    return bl


def _build_program(slot_blocks):
    """One SPMD Bass program. slot_blocks[i] is the token-block decomposition
    of slot i (fixed caps shared by all cores)."""
    import concourse.mybir as mybir
    from concourse import bacc
    from concourse.tile import TileContext

    caps = [sum(b) for b in slot_blocks]
    m = len(caps)
    seg_off = np.zeros(m + 1, np.int64)
    np.cumsum(caps, out=seg_off[1:])
    NC = int(seg_off[-1])
    bf = mybir.dt.bfloat16
    f32 = mybir.dt.float32
    Silu = mybir.ActivationFunctionType.Silu
    mult = mybir.AluOpType.mult

    # All bulk inputs are host-packed partition-major so every DMA is 128
    # contiguous runs (descriptor issue on the sequencer costs ~4.7ns/run;
    # multi-KB-per-partition transfers keep issue at ~0.6us each).
    blk_off = []  # [slot][block] -> column offset into xq
    xcols = 0
    for bl in slot_blocks:
        offs = []
        for bn in bl:
            offs.append(xcols)
            xcols += HC * bn
        blk_off.append(offs)

    nc = bacc.Bacc("TRN2", target_bir_lowering=False, debug=False, num_devices=NCORES)
    xq = nc.dram_tensor("xq", [P, xcols], bf, kind="ExternalInput").ap()
    wgu = nc.dram_tensor("wgu", [m, P, NPH * H], bf, kind="ExternalInput").ap()
    wd = nc.dram_tensor("wd", [m, P, IC * H], bf, kind="ExternalInput").ap()
    cv = nc.dram_tensor("cv", [m, P, MAXTILES], f32, kind="ExternalInput").ap()
    g = nc.dram_tensor("g", [NC, H], bf, kind="ExternalOutput").ap()

    def mm_group(tensor_eng, w_ap, mms):
        """Explicit LDWEIGHTS + weight-elided matmuls sharing it."""
        tensor_eng.ldweights(w_ap)
        for out_ap, rhs_ap, start, stop in mms:
            mm = tensor_eng.matmul(
                out=out_ap, lhsT=w_ap, rhs=rhs_ap, start=start, stop=stop
            )
            mm.ins.ldweights = False

    with TileContext(nc) as tc:
        with (
            tc.tile_pool(name="wpool", bufs=2) as wpool,
            tc.tile_pool(name="xpool", bufs=3) as xpool,
            tc.tile_pool(name="apool", bufs=2) as apool,
            tc.tile_pool(name="spool", bufs=2) as spool,
            tc.tile_pool(name="opool", bufs=6) as opool,
            tc.tile_pool(name="cpool", bufs=2) as cpool,
            tc.tile_pool(name="psgu", bufs=1, space="PSUM") as psgu,
            tc.tile_pool(name="pso", bufs=2, space="PSUM") as pso,
        ):
            wgu_r = wgu.rearrange("m p (f h) -> m p f h", f=NPH)  # [m, 128, NPH, H]
            wd_r = wd.rearrange("m p (c h) -> m p c h", c=IC)  # [m, 128, IC, H]
            pending_down = []

            def _make_down_tile(gq, q, t0, tn, act_tile, wd_tile, ct_tile):
                # gq: global output row base; q: slot-local tile index (cv col);
                # t0: token offset within the act tile's block
                def emit():
                    po = pso.tile([P, H], f32, tag="po", name="po")
                    for i in range(IC):
                        mm_group(
                            nc.tensor,
                            act_tile[:, i, t0 : t0 + tn],
                            [
                                (
                                    po[:tn, nh * 512 : (nh + 1) * 512],
                                    wd_tile[:, i, nh * 512 : (nh + 1) * 512],
                                    i == 0,
                                    i == IC - 1,
                                )
                                for nh in range(2)
                            ],
                        )
                    ob = opool.tile([P, H], bf, tag="ob", name="ob")
                    nc.vector.tensor_tensor(
                        out=ob[:tn, :],
                        in0=po[:tn, :],
                        in1=ct_tile[:tn, q : q + 1].to_broadcast([tn, H]),
                        op=mult,
                    )
                    # NOTE: must issue on sync, not scalar — a dependent DMA
                    # on the scalar queue blocks later silu instructions
                    # behind its semaphore wait and stalls the PE
                    nc.sync.dma_start(out=g[gq : gq + tn, :], in_=ob[:tn, :])

                return emit

            for ei in range(m):
                blocks = slot_blocks[ei]
                wgu_t = wpool.tile([P, NPH, H], bf, tag="wgu")
                if ei == 0:
                    # head latency: per-queue DMA bw is ~22GB/s, so the first
                    # x block (256 tokens, 512KB) and phase-0 weights (256KB)
                    # arrive ~2.5us after issue; later phases stream in chunks
                    # sized to land just before their compute
                    nc.sync.dma_start(out=wgu_t[:, 0, :], in_=wgu_r[0][:, 0, :])
                    bn0 = blocks[0]
                    xg_t = xpool.tile([P, HC * BLK], bf, tag="xg")
                    hx = HC * bn0 // 2
                    nc.sync.dma_start(out=xg_t[:, :hx], in_=xq[:, 0:hx])
                    nc.sync.dma_start(
                        out=xg_t[:, hx : HC * bn0], in_=xq[:, hx : HC * bn0]
                    )
                    nc.sync.dma_start(out=wgu_t[:, 1, :], in_=wgu_r[0][:, 1, :])
                    nc.sync.dma_start(out=wgu_t[:, 2, :], in_=wgu_r[0][:, 2, :])
                    nc.sync.dma_start(out=wgu_t[:, 3:5, :], in_=wgu_r[0][:, 3:5, :])
                    nc.sync.dma_start(out=wgu_t[:, 5:8, :], in_=wgu_r[0][:, 5:8, :])
                    nc.sync.dma_start(out=wgu_t[:, 8:, :], in_=wgu_r[0][:, 8:, :])
                    # prewarm: the PE clock boost needs ~3us of continuous
                    # execution (and resets on long idle); run a few dummy
                    # matmuls on the just-arrived phase-0 weights while
                    # block-0 x is still in flight so real matmuls start
                    # near full clock (each dummy runs ~430ns pre-boost)
                    pwarm = pso.tile([P, H], f32, tag="po", name="po")
                    for _ in range(5):
                        mm_group(
                            nc.tensor,
                            wgu_t[:, 0, 0:P],
                            [(pwarm[:, :512], wgu_t[:, 0, 0:512], True, True)],
                        )
                else:
                    nc.sync.dma_start(out=wgu_t[:], in_=wgu_r[ei])
                wd_t = wpool.tile([P, IC, H], bf, tag="wd")
                nc.sync.dma_start(out=wd_t[:], in_=wd_r[ei])
                ct = cpool.tile([P, MAXTILES], f32, tag="ct")
                nc.sync.dma_start(out=ct[:], in_=cv[ei])

                off = 0
                for bi, bn in enumerate(blocks):
                    s = int(seg_off[ei]) + off
                    if not (ei == 0 and bi == 0):
                        xg_t = xpool.tile([P, HC * BLK], bf, tag="xg")
                        bo = blk_off[ei][bi]
                        nc.sync.dma_start(
                            out=xg_t[:, : HC * bn], in_=xq[:, bo : bo + HC * bn]
                        )
                    # sub-blocks of <=512, smallest first: the LAST matmul of
                    # each weight group must be wide enough to hide the next
                    # group's LDWEIGHTS behind its streaming
                    sbs = sorted(
                        (
                            (qq * 512, min(512, bn - qq * 512))
                            for qq in range((bn + 511) // 512)
                        ),
                        key=lambda t: t[1],
                    )
                    act_sb = apool.tile([P, IC, BLK], bf, tag="act")
                    # gate/up phase pairs with the previous block's down tiles
                    # interleaved between pairs (stretches every PSUM-reuse
                    # distance past the ACT/DVE consumer chain)
                    ndp = len(pending_down)
                    emitted = 0
                    for jj in range(IC):
                        pg = [
                            psgu.tile([P, 512], f32, tag=f"pg{si}", name=f"pg{si}")
                            for si in range(len(sbs))
                        ]
                        pu = [
                            psgu.tile([P, 512], f32, tag=f"pu{si}", name=f"pu{si}")
                            for si in range(len(sbs))
                        ]
                        for gi, ps_tiles in ((0, pg), (1, pu)):
                            ph = 2 * jj + gi
                            for hc in range(HC):
                                mm_group(
                                    nc.tensor,
                                    wgu_t[:, ph, hc * P : (hc + 1) * P],
                                    [
                                        (
                                            ps_tiles[si][:, :qn],
                                            xg_t[:, hc * bn + q0 : hc * bn + q0 + qn],
                                            hc == 0,
                                            hc == HC - 1,
                                        )
                                        for si, (q0, qn) in enumerate(sbs)
                                    ],
                                )
                        for si, (q0, qn) in enumerate(sbs):
                            sg = spool.tile([P, 512], f32, tag=f"sg{si}", name=f"sg{si}")
                            nc.scalar.activation(
                                out=sg[:, :qn], in_=pg[si][:, :qn], func=Silu
                            )
                            nc.vector.tensor_tensor(
                                out=act_sb[:, jj, q0 : q0 + qn],
                                in0=sg[:, :qn],
                                in1=pu[si][:, :qn],
                                op=mult,
                            )
                        target = (jj + 1) * ndp // IC
                        while emitted < target:
                            pending_down.pop(0)()
                            emitted += 1
                    nt = (bn + P - 1) // P
                    for ts in range(nt):
                        pending_down.append(
                            _make_down_tile(
                                s + ts * P,
                                (off + ts * P) // P,
                                ts * P,
                                min(P, bn - ts * P),
                                act_sb,
                                wd_t,
                                ct,
                            )
                        )
                    off += bn
            while pending_down:
                pending_down.pop(0)()
    nc.compile()
    return nc


def kernel(hidden_states, gate_weight, correction_bias, w_gate, w_up, w_down):
    global last_results
    from concourse.bass_utils import run_bass_kernel_spmd

    hidden = np.ascontiguousarray(np.asarray(hidden_states, np.float32))
    w_gate = np.asarray(w_gate, np.float32)
    w_up = np.asarray(w_up, np.float32)
    w_down = np.asarray(w_down, np.float32)

    topk_idx, topk_w = _routing(hidden, gate_weight, correction_bias)

    # Per-expert token lists (ascending), via stable sort of the (token, k) pairs.
    flat_e = topk_idx.ravel()
    order = np.argsort(flat_e, kind="stable")
    tokens_sorted = (order // K).astype(np.int64)
    weights_sorted = topk_w.ravel()[order]
    counts = np.bincount(flat_e, minlength=E)
    starts = np.zeros(E + 1, np.int64)
    np.cumsum(counts, out=starts[1:])

    # Pieces -> rank groups of 8 -> slots; big/small interleaved slot order.
    eps = _pack_pieces(counts)
    pieces = []  # (size, expert, offset within expert token list)
    for e, ps in enumerate(eps):
        off = 0
        for p in sorted(ps, reverse=True):
            pieces.append((int(p), e, off))
            off += p
    pieces.sort(key=lambda t: (-t[0], t[1], t[2]))
    assert len(pieces) % 8 == 0
    nrank = len(pieces) // 8
    half = (nrank + 1) // 2
    rank_order = []
    for i in range(half):
        rank_order.append(i)
        if i + half < nrank:
            rank_order.append(i + half)
    # slot i <- rank rank_order[i]
    slot_pieces = [pieces[8 * r : 8 * r + 8] for r in rank_order]
    caps = [grp[0][0] for grp in slot_pieces]
    m = len(caps)
    slot_blocks = tuple(
        tuple(_blocks_of(int(caps[i]), warm=(i == 0))) for i in range(m)
    )
    seg_off = np.zeros(m + 1, np.int64)
    np.cumsum(caps, out=seg_off[1:])
    NC = int(seg_off[-1])

    print(
        f"[kernel] counts min/mean/max: {counts.min()}/{counts.mean():.0f}/{counts.max()}; "
        f"m={m} sumcaps {NC} pad {8 * NC - int(counts.sum())}"
    )
    if slot_blocks not in _program_cache:
        _program_cache[slot_blocks] = _build_program([list(b) for b in slot_blocks])
    nc = _program_cache[slot_blocks]

    # per-expert phase-major weight arrays (cached across calls by id fingerprint)
    wkey = (
        float(w_gate[0, 0, 0]),
        float(w_up[0, 0, 0]),
        float(w_down[-1, -1, -1]),
        w_gate.shape,
    )
    cached = _weights_cache.get(wkey)
    if cached is None:
        # partition-major: wgu_e[e, p, ph*H + hc*128 + c], ph = 2*jj + (0 g/1 u)
        wgu_e = np.empty((E, P, NPH, H), BF16)
        wd_e = np.empty((E, P, IC, H), BF16)
        for e in range(E):
            gp = (
                w_gate[e].reshape(IC, P, HC, P).transpose(3, 0, 2, 1).reshape(P, IC, H)
            )  # [p, jj, hc*128+c] = wg[jj*128+c, hc*128+p]
            up = w_up[e].reshape(IC, P, HC, P).transpose(3, 0, 2, 1).reshape(P, IC, H)
            wgu_e[e, :, 0::2] = gp.astype(BF16)
            wgu_e[e, :, 1::2] = up.astype(BF16)
            # wd_e[e, p, ic*H + h] = w_down[e][h, ic*128+p]
            wd_e[e] = (
                w_down[e].T.reshape(IC, P, H).transpose(1, 0, 2).astype(BF16)
            )
        wgu_e = wgu_e.reshape(E, P, NPH * H)
        wd_e = wd_e.reshape(E, P, IC * H)
        _weights_cache.clear()
        cached = (wgu_e, wd_e)
        _weights_cache[wkey] = cached
    wgu_e, wd_e = cached

    # x block layout: per (slot, block), partition-major [128, HC*bn] columns
    blk_off = []
    xcols = 0
    for bl in slot_blocks:
        offs = []
        for bn in bl:
            offs.append(xcols)
            xcols += HC * bn
        blk_off.append(offs)

    hidden_bf_t = np.ascontiguousarray(hidden.T).astype(BF16)  # [H, T]
    hbt3 = hidden_bf_t.reshape(HC, P, T)
    in_maps = []
    core_slot_info = []  # [core][slot] = (expert, n, token array)
    for c in range(NCORES):
        slot_exp = np.empty(m, np.int64)
        perm = np.zeros(NC, np.int64)
        cvh = np.zeros((m, P, MAXTILES), np.float32)
        info = []
        for i, grp in enumerate(slot_pieces):
            pn, e, poff = grp[c]
            slot_exp[i] = e
            s = int(seg_off[i])
            te = tokens_sorted[starts[e] + poff : starts[e] + poff + pn]
            perm[s : s + pn] = te
            wv = weights_sorted[starts[e] + poff : starts[e] + poff + pn]
            wpad = np.zeros(P * MAXTILES, np.float32)
            wpad[:pn] = wv
            cvh[i] = wpad.reshape(MAXTILES, P).T
            info.append((e, pn, te))
        core_slot_info.append(info)
        xqc = np.empty((P, xcols), BF16)
        for i in range(m):
            s = int(seg_off[i])
            off = 0
            for bi, bn in enumerate(slot_blocks[i]):
                bo = blk_off[i][bi]
                blkx = hbt3[:, :, perm[s + off : s + off + bn]]  # [HC, P, bn]
                xqc[:, bo : bo + HC * bn] = blkx.transpose(1, 0, 2).reshape(
                    P, HC * bn
                )
                off += bn
        in_maps.append(
            {
                "xq": xqc,
                "wgu": wgu_e[slot_exp],
                "wd": wd_e[slot_exp],
                "cv": cvh,
            }
        )

    last_results = run_bass_kernel_spmd(nc, in_maps, list(range(NCORES)))

    out = np.zeros((T, H), np.float32)
    for c in range(NCORES):
        gc = last_results.results[c]["g"]
        for i in range(m):
            e, pn, te = core_slot_info[c][i]
            s = int(seg_off[i])
            out[te] += gc[s : s + pn].astype(np.float32)
    return out


# revision 22
# speedup vs baseline: 1.0078x; 1.0033x over previous
"""MoE (MiMo-V2) kernel for 8x Trainium2 NeuronCores.

Strategy (expert-parallel with expert splitting):
  - Host: grouped-topk routing (exact replica of the reference gate, fp32 on
    jax-cpu). Experts are split into pieces (a piece = contiguous run of one
    expert's tokens); pieces are rank-grouped 8-at-a-time into SPMD slots with
    shared caps. Piece sizes are optimized (structured init + hill climb) to
    minimize PE cycles: f = 96*sum(caps) + 6144*sum(ceil(cap/128)), i.e.
    gate/up columns plus fixed-cost 128-token down tiles. Splitting lets caps
    hug the counts (padding ~1%o vs ~2.7% for whole-expert assignment).
  - Device (Bass/Tile, one SPMD program): per slot, stream token blocks of
    <=1024 through gate/up matmuls (bf16, fp32 PSUM, explicit LDWEIGHTS with
    weight-elided matmuls), silu*mul on ACT/DVE into an [I, tokens] act tile;
    the previous block's down matmuls are interleaved between gate/up chunk
    pairs; down rows are scaled by combine weights and written out in bf16.
    Weights are stored phase-major ([12, 128, 1024] per expert) so the first
    matmul needs only one 256KB transfer; each x block is a single 3D-AP DMA
    (the DMA-trigger issue rate on the sync sequencer, ~0.6us each, is a
    warmup bottleneck). Slots are ordered big/small interleaved so small
    slots' weights prefetch under big slots' compute.
  - Host: scatter-add the per-piece rows into the [T, H] output.
"""

import numpy as np
import ml_dtypes

T, H, E, I, K, G, KG = 16384, 1024, 64, 768, 8, 8, 4
P = 128
NCORES = 8
HC = H // P  # 8 contraction chunks for gate/up
IC = I // P  # 6 contraction chunks for down (also the jj phase count)
NPH = 2 * IC  # 12 gate/up phases (gate jj, up jj)
BLK = 1024  # token block (<=2 PSUM sub-blocks of 512 per phase)
MAXTILES = 8  # down tiles per slot (cap <= 1024)

BF16 = ml_dtypes.bfloat16

_program_cache = {}
_weights_cache = {}
last_results = None  # BassKernelResults of the most recent launch (for test.py)


def _routing_np(hidden, gate_w, bias):
    """Numpy fallback for the grouped-topk gate (same ops/tie rules)."""
    logits = hidden.astype(np.float32) @ gate_w.T.astype(np.float32)
    scores = 1.0 / (1.0 + np.exp(-logits))
    s_choice = scores + bias[None, :].astype(np.float32)
    t, e = scores.shape
    grouped = s_choice.reshape(t, G, e // G)
    top2 = np.sort(grouped, axis=-1)[..., -2:]
    group_scores = top2.sum(-1)
    gidx = np.argsort(-group_scores, axis=1, kind="stable")[:, :KG]
    gmask = np.zeros((t, G), np.float32)
    gmask[np.arange(t)[:, None], gidx] = 1.0
    emask = np.repeat(gmask, e // G, axis=1)
    masked = np.where(emask > 0, s_choice, -np.inf)
    topk_idx = np.argsort(-masked, axis=1, kind="stable")[:, :K].astype(np.int32)
    topk_w = np.take_along_axis(scores, topk_idx, axis=1)
    topk_w = topk_w / (topk_w.sum(-1, keepdims=True) + 1e-20)
    return topk_idx, topk_w.astype(np.float32)


def _routing(hidden, gate_w, bias):
    """Exact replica of reference._grouped_topk on jax-cpu (fp32)."""
    try:
        import jax
        import jax.numpy as jnp

        cpu = jax.devices("cpu")[0]
    except Exception:
        return _routing_np(np.asarray(hidden), np.asarray(gate_w), np.asarray(bias))
    with jax.default_device(cpu):
        hidden = jnp.asarray(np.asarray(hidden), jnp.float32)
        gate_w = jnp.asarray(np.asarray(gate_w), jnp.float32)
        bias = jnp.asarray(np.asarray(bias), jnp.float32)
        logits = hidden @ gate_w.T
        scores = jax.nn.sigmoid(logits)
        s_choice = scores + bias[None, :]
        t, e = scores.shape
        grouped = s_choice.reshape(t, G, e // G)
        top2, _ = jax.lax.top_k(grouped, 2)
        group_scores = top2.sum(-1)
        _, gidx = jax.lax.top_k(group_scores, KG)
        gmask = jnp.zeros((t, G), jnp.float32).at[jnp.arange(t)[:, None], gidx].set(1.0)
        emask = jnp.repeat(gmask, e // G, axis=1)
        masked = jnp.where(emask > 0, s_choice, -jnp.inf)
        _, topk_idx = jax.lax.top_k(masked, K)
        topk_w = jnp.take_along_axis(scores, topk_idx, axis=1)
        topk_w = topk_w / (topk_w.sum(-1, keepdims=True) + 1e-20)
        return np.asarray(topk_idx), np.asarray(topk_w, np.float32)


def _pack_pieces(counts, iters=240000, seed=1):
    """Split expert counts into pieces so that the sorted piece multiset
    rank-groups (8 at a time) into slots with minimal PE cost.

    Cost per core: f = 96*sum(caps) + 6144*sum(ceil(caps/128)) cycles
    (gate/up columns + fixed-size 128-token down tiles). Structured init:
    one ragged piece of 384+(c%128) per expert (ragged caps cluster by
    c%128), remainder in 128-multiple pieces of <=1024. A hill climb with
    128-granular and fine moves between same-expert pieces cleans up.
    Returns eps: list (per expert) of piece sizes.
    """
    import random

    rng = random.Random(seed)
    counts = [int(c) for c in counts]
    eps = []
    for c in counts:
        if c <= 0:
            eps.append([])
            continue
        if c < 896:
            eps.append([c])
            continue
        rag = 384 + (c % 128)
        n = (c - rag) // 128
        ps = [rag]
        while n > 8:
            take = min(8, n - 4)
            ps.append(128 * take)
            n -= take
        if n:
            ps.append(128 * n)
        eps.append(ps)

    def nflat():
        return sum(len(ps) for ps in eps)

    while nflat() % 8:
        bi, bj = max(
            ((i, j) for i, ps in enumerate(eps) for j in range(len(ps))),
            key=lambda t: eps[t[0]][t[1]],
        )
        p = eps[bi].pop(bj)
        h = max(128, (p // 2) // 128 * 128)
        eps[bi] += [p - h, h]

    def obj():
        flat = sorted((p for ps in eps for p in ps), reverse=True)
        caps = flat[0::8]
        fv = 96 * sum(caps) + 6144 * sum(-(-c // 128) for c in caps)
        if flat[-1] < 384:
            fv += 1_000_000
        return fv

    cur = obj()
    movers = [i for i, ps in enumerate(eps) if len(ps) >= 2]
    if movers:
        for _ in range(iters):
            i = movers[rng.randrange(len(movers))]
            ps = eps[i]
            a = rng.randrange(len(ps))
            b = rng.randrange(len(ps))
            if a == b:
                continue
            d = rng.choice((1, 2, 4, 8, 16, 32, 64, 128, 256))
            if ps[a] - d < 128 or ps[b] + d > BLK:
                continue
            ps[a] -= d
            ps[b] += d
            f2 = obj()
            if f2 <= cur:
                cur = f2
            else:
                ps[a] += d
                ps[b] -= d
    return eps


def _blocks_of(cap, warm=False):
    """Decompose a slot capacity into token blocks of <=BLK.

    All blocks except the last are multiples of 128 (keeps down tiles
    128-aligned within the slot). warm=True (first slot) starts with a
    512-token block for a short first-weights transfer and early PE ramp.
    """
    bl = []
    work = cap
    if warm and cap >= 896:
        bl.append(512)
        work -= 512
    nb = -(-work // BLK)
    base = int(round(work / nb / P)) * P
    while work - base * (nb - 1) > BLK:
        base += P
    while nb > 1 and work - base * (nb - 1) <= 0:
        base -= P
    bl += [base] * (nb - 1) + [work - base * (nb - 1)]
    assert all(0 < b <= BLK for b in bl) and sum(bl) == cap, (cap, bl)
    return bl


def _build_program(slot_blocks):
    """One SPMD Bass program. slot_blocks[i] is the token-block decomposition
    of slot i (fixed caps shared by all cores)."""
    import concourse.mybir as mybir
    from concourse import bacc
    from concourse.tile import TileContext

    caps = [sum(b) for b in slot_blocks]
    m = len(caps)
    seg_off = np.zeros(m + 1, np.int64)
    np.cumsum(caps, out=seg_off[1:])
    NC = int(seg_off[-1])
    bf = mybir.dt.bfloat16
    f32 = mybir.dt.float32
    Silu = mybir.ActivationFunctionType.Silu
    mult = mybir.AluOpType.mult

    # All bulk inputs are host-packed partition-major so every DMA is 128
    # contiguous runs (descriptor issue on the sequencer costs ~4.7ns/run;
    # multi-KB-per-partition transfers keep issue at ~0.6us each).
    blk_off = []  # [slot][block] -> column offset into xq
    xcols = 0
    for bl in slot_blocks:
        offs = []
        for bn in bl:
            offs.append(xcols)
            xcols += HC * bn
        blk_off.append(offs)

    nc = bacc.Bacc("TRN2", target_bir_lowering=False, debug=False, num_devices=NCORES)
    xq = nc.dram_tensor("xq", [P, xcols], bf, kind="ExternalInput").ap()
    wgu = nc.dram_tensor("wgu", [m, P, NPH * H], bf, kind="ExternalInput").ap()
    wd = nc.dram_tensor("wd", [m, P, IC * H], bf, kind="ExternalInput").ap()
    cv = nc.dram_tensor("cv", [m, P, MAXTILES], f32, kind="ExternalInput").ap()
    g = nc.dram_tensor("g", [NC, H], bf, kind="ExternalOutput").ap()

    def mm_group(tensor_eng, w_ap, mms):
        """Explicit LDWEIGHTS + weight-elided matmuls sharing it."""
        tensor_eng.ldweights(w_ap)
        for out_ap, rhs_ap, start, stop in mms:
            mm = tensor_eng.matmul(
                out=out_ap, lhsT=w_ap, rhs=rhs_ap, start=start, stop=stop
            )
            mm.ins.ldweights = False

    with TileContext(nc) as tc:
        with (
            tc.tile_pool(name="wpool", bufs=2) as wpool,
            tc.tile_pool(name="xpool", bufs=3) as xpool,
            tc.tile_pool(name="apool", bufs=2) as apool,
            tc.tile_pool(name="spool", bufs=2) as spool,
            tc.tile_pool(name="opool", bufs=6) as opool,
            tc.tile_pool(name="cpool", bufs=2) as cpool,
            tc.tile_pool(name="psgu", bufs=1, space="PSUM") as psgu,
            tc.tile_pool(name="pso", bufs=2, space="PSUM") as pso,
        ):
            wgu_r = wgu.rearrange("m p (f h) -> m p f h", f=NPH)  # [m, 128, NPH, H]
            wd_r = wd.rearrange("m p (c h) -> m p c h", c=IC)  # [m, 128, IC, H]
            pending_down = []

            def _make_down_tile(gq, q, t0, tn, act_tile, wd_tile, ct_tile):
                # gq: global output row base; q: slot-local tile index (cv col);
                # t0: token offset within the act tile's block
                def emit():
                    po = pso.tile([P, H], f32, tag="po", name="po")
                    for i in range(IC):
                        mm_group(
                            nc.tensor,
                            act_tile[:, i, t0 : t0 + tn],
                            [
                                (
                                    po[:tn, nh * 512 : (nh + 1) * 512],
                                    wd_tile[:, i, nh * 512 : (nh + 1) * 512],
                                    i == 0,
                                    i == IC - 1,
                                )
                                for nh in range(2)
                            ],
                        )
                    ob = opool.tile([P, H], bf, tag="ob", name="ob")
                    nc.vector.tensor_tensor(
                        out=ob[:tn, :],
                        in0=po[:tn, :],
                        in1=ct_tile[:tn, q : q + 1].to_broadcast([tn, H]),
                        op=mult,
                    )
                    # NOTE: must issue on sync, not scalar — a dependent DMA
                    # on the scalar queue blocks later silu instructions
                    # behind its semaphore wait and stalls the PE
                    nc.sync.dma_start(out=g[gq : gq + tn, :], in_=ob[:tn, :])

                return emit

            for ei in range(m):
                blocks = slot_blocks[ei]
                wgu_t = wpool.tile([P, NPH, H], bf, tag="wgu")
                if ei == 0:
                    # head latency: per-queue DMA bw is ~22GB/s, so the first
                    # x block (256 tokens, 512KB) and phase-0 weights (256KB)
                    # arrive ~2.5us after issue; later phases stream in chunks
                    # sized to land just before their compute
                    nc.sync.dma_start(out=wgu_t[:, 0, :], in_=wgu_r[0][:, 0, :])
                    bn0 = blocks[0]
                    xg_t = xpool.tile([P, HC * BLK], bf, tag="xg")
                    hx = HC * bn0 // 2
                    nc.sync.dma_start(out=xg_t[:, :hx], in_=xq[:, 0:hx])
                    nc.sync.dma_start(
                        out=xg_t[:, hx : HC * bn0], in_=xq[:, hx : HC * bn0]
                    )
                    nc.sync.dma_start(out=wgu_t[:, 1, :], in_=wgu_r[0][:, 1, :])
                    nc.sync.dma_start(out=wgu_t[:, 2, :], in_=wgu_r[0][:, 2, :])
                    nc.sync.dma_start(out=wgu_t[:, 3:5, :], in_=wgu_r[0][:, 3:5, :])
                    nc.sync.dma_start(out=wgu_t[:, 5:8, :], in_=wgu_r[0][:, 5:8, :])
                    nc.sync.dma_start(out=wgu_t[:, 8:, :], in_=wgu_r[0][:, 8:, :])
                    # prewarm: the PE clock boost needs ~3us of continuous
                    # execution (and resets on long idle); run a few dummy
                    # matmuls on the just-arrived phase-0 weights while
                    # block-0 x is still in flight so real matmuls start
                    # near full clock (each dummy runs ~430ns pre-boost)
                    pwarm = pso.tile([P, H], f32, tag="po", name="po")
                    for _ in range(6):
                        mm_group(
                            nc.tensor,
                            wgu_t[:, 0, 0:P],
                            [(pwarm[:, :512], wgu_t[:, 0, 0:512], True, True)],
                        )
                else:
                    nc.sync.dma_start(out=wgu_t[:], in_=wgu_r[ei])
                wd_t = wpool.tile([P, IC, H], bf, tag="wd")
                nc.sync.dma_start(out=wd_t[:], in_=wd_r[ei])
                ct = cpool.tile([P, MAXTILES], f32, tag="ct")
                nc.sync.dma_start(out=ct[:], in_=cv[ei])

                off = 0
                for bi, bn in enumerate(blocks):
                    s = int(seg_off[ei]) + off
                    if not (ei == 0 and bi == 0):
                        xg_t = xpool.tile([P, HC * BLK], bf, tag="xg")
                        bo = blk_off[ei][bi]
                        nc.sync.dma_start(
                            out=xg_t[:, : HC * bn], in_=xq[:, bo : bo + HC * bn]
                        )
                    # sub-blocks of <=512, smallest first: the LAST matmul of
                    # each weight group must be wide enough to hide the next
                    # group's LDWEIGHTS behind its streaming
                    sbs = sorted(
                        (
                            (qq * 512, min(512, bn - qq * 512))
                            for qq in range((bn + 511) // 512)
                        ),
                        key=lambda t: t[1],
                    )
                    act_sb = apool.tile([P, IC, BLK], bf, tag="act")
                    # gate/up phase pairs with the previous block's down tiles
                    # interleaved between pairs (stretches every PSUM-reuse
                    # distance past the ACT/DVE consumer chain)
                    ndp = len(pending_down)
                    emitted = 0
                    for jj in range(IC):
                        pg = [
                            psgu.tile([P, 512], f32, tag=f"pg{si}", name=f"pg{si}")
                            for si in range(len(sbs))
                        ]
                        pu = [
                            psgu.tile([P, 512], f32, tag=f"pu{si}", name=f"pu{si}")
                            for si in range(len(sbs))
                        ]
                        for gi, ps_tiles in ((0, pg), (1, pu)):
                            ph = 2 * jj + gi
                            for hc in range(HC):
                                mm_group(
                                    nc.tensor,
                                    wgu_t[:, ph, hc * P : (hc + 1) * P],
                                    [
                                        (
                                            ps_tiles[si][:, :qn],
                                            xg_t[:, hc * bn + q0 : hc * bn + q0 + qn],
                                            hc == 0,
                                            hc == HC - 1,
                                        )
                                        for si, (q0, qn) in enumerate(sbs)
                                    ],
                                )
                        for si, (q0, qn) in enumerate(sbs):
                            sg = spool.tile([P, 512], f32, tag=f"sg{si}", name=f"sg{si}")
                            nc.scalar.activation(
                                out=sg[:, :qn], in_=pg[si][:, :qn], func=Silu
                            )
                            nc.vector.tensor_tensor(
                                out=act_sb[:, jj, q0 : q0 + qn],
                                in0=sg[:, :qn],
                                in1=pu[si][:, :qn],
                                op=mult,
                            )
                        target = (jj + 1) * ndp // IC
                        while emitted < target:
                            pending_down.pop(0)()
                            emitted += 1
                    nt = (bn + P - 1) // P
                    for ts in range(nt):
                        pending_down.append(
                            _make_down_tile(
                                s + ts * P,
                                (off + ts * P) // P,
                                ts * P,
                                min(P, bn - ts * P),
                                act_sb,
                                wd_t,
                                ct,
                            )
                        )
                    off += bn
            while pending_down:
                pending_down.pop(0)()
    nc.compile()
    return nc


def kernel(hidden_states, gate_weight, correction_bias, w_gate, w_up, w_down):
    global last_results
    from concourse.bass_utils import run_bass_kernel_spmd

    hidden = np.ascontiguousarray(np.asarray(hidden_states, np.float32))
    w_gate = np.asarray(w_gate, np.float32)
    w_up = np.asarray(w_up, np.float32)
    w_down = np.asarray(w_down, np.float32)

    topk_idx, topk_w = _routing(hidden, gate_weight, correction_bias)

    # Per-expert token lists (ascending), via stable sort of the (token, k) pairs.
    flat_e = topk_idx.ravel()
    order = np.argsort(flat_e, kind="stable")
    tokens_sorted = (order // K).astype(np.int64)
    weights_sorted = topk_w.ravel()[order]
    counts = np.bincount(flat_e, minlength=E)
    starts = np.zeros(E + 1, np.int64)
    np.cumsum(counts, out=starts[1:])

    # Pieces -> rank groups of 8 -> slots; big/small interleaved slot order.
    eps = _pack_pieces(counts)
    pieces = []  # (size, expert, offset within expert token list)
    for e, ps in enumerate(eps):
        off = 0
        for p in sorted(ps, reverse=True):
            pieces.append((int(p), e, off))
            off += p
    pieces.sort(key=lambda t: (-t[0], t[1], t[2]))
    assert len(pieces) % 8 == 0
    nrank = len(pieces) // 8
    half = (nrank + 1) // 2
    rank_order = []
    for i in range(half):
        rank_order.append(i)
        if i + half < nrank:
            rank_order.append(i + half)
    # slot i <- rank rank_order[i]
    slot_pieces = [pieces[8 * r : 8 * r + 8] for r in rank_order]
    caps = [grp[0][0] for grp in slot_pieces]
    m = len(caps)
    slot_blocks = tuple(
        tuple(_blocks_of(int(caps[i]), warm=(i == 0))) for i in range(m)
    )
    seg_off = np.zeros(m + 1, np.int64)
    np.cumsum(caps, out=seg_off[1:])
    NC = int(seg_off[-1])

    print(
        f"[kernel] counts min/mean/max: {counts.min()}/{counts.mean():.0f}/{counts.max()}; "
        f"m={m} sumcaps {NC} pad {8 * NC - int(counts.sum())}"
    )
    if slot_blocks not in _program_cache:
        _program_cache[slot_blocks] = _build_program([list(b) for b in slot_blocks])
    nc = _program_cache[slot_blocks]

    # per-expert phase-major weight arrays (cached across calls by id fingerprint)
    wkey = (
        float(w_gate[0, 0, 0]),
        float(w_up[0, 0, 0]),
        float(w_down[-1, -1, -1]),
        w_gate.shape,
    )
    cached = _weights_cache.get(wkey)
    if cached is None:
        # partition-major: wgu_e[e, p, ph*H + hc*128 + c], ph = 2*jj + (0 g/1 u)
        wgu_e = np.empty((E, P, NPH, H), BF16)
        wd_e = np.empty((E, P, IC, H), BF16)
        for e in range(E):
            gp = (
                w_gate[e].reshape(IC, P, HC, P).transpose(3, 0, 2, 1).reshape(P, IC, H)
            )  # [p, jj, hc*128+c] = wg[jj*128+c, hc*128+p]
            up = w_up[e].reshape(IC, P, HC, P).transpose(3, 0, 2, 1).reshape(P, IC, H)
            wgu_e[e, :, 0::2] = gp.astype(BF16)
            wgu_e[e, :, 1::2] = up.astype(BF16)
            # wd_e[e, p, ic*H + h] = w_down[e][h, ic*128+p]
            wd_e[e] = (
                w_down[e].T.reshape(IC, P, H).transpose(1, 0, 2).astype(BF16)
            )
        wgu_e = wgu_e.reshape(E, P, NPH * H)
        wd_e = wd_e.reshape(E, P, IC * H)
        _weights_cache.clear()
        cached = (wgu_e, wd_e)
        _weights_cache[wkey] = cached
    wgu_e, wd_e = cached

    # x block layout: per (slot, block), partition-major [128, HC*bn] columns
    blk_off = []
    xcols = 0
    for bl in slot_blocks:
        offs = []
        for bn in bl:
            offs.append(xcols)
            xcols += HC * bn
        blk_off.append(offs)

    hidden_bf_t = np.ascontiguousarray(hidden.T).astype(BF16)  # [H, T]
    hbt3 = hidden_bf_t.reshape(HC, P, T)
    in_maps = []
    core_slot_info = []  # [core][slot] = (expert, n, token array)
    for c in range(NCORES):
        slot_exp = np.empty(m, np.int64)
        perm = np.zeros(NC, np.int64)
        cvh = np.zeros((m, P, MAXTILES), np.float32)
        info = []
        for i, grp in enumerate(slot_pieces):
            pn, e, poff = grp[c]
            slot_exp[i] = e
            s = int(seg_off[i])
            te = tokens_sorted[starts[e] + poff : starts[e] + poff + pn]
            perm[s : s + pn] = te
            wv = weights_sorted[starts[e] + poff : starts[e] + poff + pn]
            wpad = np.zeros(P * MAXTILES, np.float32)
            wpad[:pn] = wv
            cvh[i] = wpad.reshape(MAXTILES, P).T
            info.append((e, pn, te))
        core_slot_info.append(info)
        xqc = np.empty((P, xcols), BF16)
        for i in range(m):
            s = int(seg_off[i])
            off = 0
            for bi, bn in enumerate(slot_blocks[i]):
                bo = blk_off[i][bi]
                blkx = hbt3[:, :, perm[s + off : s + off + bn]]  # [HC, P, bn]
                xqc[:, bo : bo + HC * bn] = blkx.transpose(1, 0, 2).reshape(
                    P, HC * bn
                )
                off += bn
        in_maps.append(
            {
                "xq": xqc,
                "wgu": wgu_e[slot_exp],
                "wd": wd_e[slot_exp],
                "cv": cvh,
            }
        )

    last_results = run_bass_kernel_spmd(nc, in_maps, list(range(NCORES)))

    out = np.zeros((T, H), np.float32)
    for c in range(NCORES):
        gc = last_results.results[c]["g"]
        for i in range(m):
            e, pn, te = core_slot_info[c][i]
            s = int(seg_off[i])
            out[te] += gc[s : s + pn].astype(np.float32)
    return out
